# revision 11
# baseline (speedup 1.0000x reference)
"""LocalRNN Trainium2 kernel.

Reference computation (per batch element):
    px = (x @ Wx)                        # [S, H], then left-pad W-1 zeros in s
    state = 0
    for i in 0..W-1:
        inp  = px shifted right by (W-1-i) positions (zeros shifted in)
        ns   = state @ Wy + by           # [S, 2H]
        cand, gl = split(ns, 2, -1)
        gate = clip(1.2*sigmoid(gl) - 0.1, 0, 1)
        state = relu(gate*(inp + cand) + (1-gate)*state)
    return state                         # [S, H]

Strategy: data-parallel over batch (B=8 -> one batch element per core,
weights replicated, no collectives). On-core everything is kept in a
TRANSPOSED layout (H on SBUF partitions, S on the free dim) so the serial
window recurrence needs no per-step transposes:
    ns^T = Wy^T @ state^T    (PE: lhsT = Wy as stored, rhs = state^T)
The shifted input is a column slice of a zero-padded px^T tile.
Matmuls run in bf16 (fp32 PSUM accumulate); the fp32 state master is kept
in SBUF and a bf16 copy is refreshed each step for the next matmul.
"""

import numpy as np
import ml_dtypes

import concourse.bacc as bacc
import concourse.bass as bass
import concourse.mybir as mybir
import concourse.tile as tile
from concourse import bass_utils

F32 = mybir.dt.float32
BF16 = mybir.dt.bfloat16
AF = mybir.ActivationFunctionType
OP = mybir.AluOpType

# Problem dims (hardcoded per the spec)
B, S, H, W = 8, 2048, 1024, 16
PAD = 16            # left zero-pad of px^T (>= W-1)
NCH = 2             # column chunks per step (pipelining + in-place safety)
NS = 512            # matmul moving-operand tile (one PSUM bank of fp32)


def emit(nc, tc, *, s, h, w, nch, ns, xT, wx_d, wy_d, byt_d, p0_d, q0_d, out_d):
    """Emit the single-core program. All dims parameterizable for testing."""
    KT = h // 128          # k-tiles over H (also the number of h state tiles)
    HT2 = 2 * h // 128     # m-tiles over 2H
    CW = s // nch          # columns per chunk
    NT = max(CW // ns, 1)  # matmul n-tiles per chunk
    ns_ = min(ns, CW)
    PXW = PAD + s          # per-h-chunk width of padded px^T

    pers = tc.alloc_tile_pool(name="pers", bufs=1)
    # bf16 state, double-buffered: step i reads sb[i%2], writes sb[(i+1)%2]
    # (in-step writes must not alias the operand every m-tile matmul reads)
    sb0 = pers.tile([128, KT * s], BF16, tag="sb0")
    sb1 = pers.tile([128, KT * s], BF16, tag="sb1")
    sbufs = [sb0, sb1]
    pxT = pers.tile([128, KT * PXW], BF16, tag="pxT")
    wy = pers.tile([128, KT * 2 * h], BF16, tag="wy")
    byt = pers.tile([128, HT2], F32, tag="byt")
    p0 = pers.tile([128, KT], F32, tag="p0")
    q0 = pers.tile([128, KT], F32, tag="q0")
    cneg = pers.tile([128, 1], F32, tag="cneg")
    nc.vector.memset(cneg[:, :], -0.1)

    # --- load weights / biases -------------------------------------------
    for k in range(KT):
        nc.sync.dma_start(wy[:, k * 2 * h:(k + 1) * 2 * h],
                          wy_d[k * 128:(k + 1) * 128, :])
    nc.sync.dma_start(byt[:, :], byt_d[:, :])
    nc.sync.dma_start(p0[:, :], p0_d[:, :])
    nc.sync.dma_start(q0[:, :], q0_d[:, :])

    # zero the left pads of px^T
    for k in range(KT):
        nc.vector.memset(pxT[:, k * PXW:k * PXW + PAD], 0.0)

    # --- proj phase: px^T = Wx^T @ x^T ------------------------------------
    # x^T is streamed from DRAM in [128, ns] tiles; Wx kept resident.
    PNT = s // ns_        # n-tiles over the full S
    with tc.tile_pool(name="proj", bufs=1) as projp, \
         tc.tile_pool(name="projps", bufs=min(2 * KT, 8), space="PSUM") as projps, \
         tc.tile_pool(name="xs", bufs=3) as xsp:
        wx = projp.tile([128, KT * h], BF16, tag="wx")
        for k in range(KT):
            nc.sync.dma_start(wx[:, k * h:(k + 1) * h],
                              wx_d[k * 128:(k + 1) * 128, :])
        for n in range(PNT):
            pp = [projps.tile([128, ns_], F32, tag="pp", name=f"pp{n}_{m}")
                  for m in range(KT)]
            for k in range(KT):
                xn = xsp.tile([128, ns_], BF16, tag="xn")
                nc.sync.dma_start(
                    xn[:, :], xT[k * 128:(k + 1) * 128, n * ns_:(n + 1) * ns_])
                for m in range(KT):
                    nc.tensor.matmul(
                        pp[m][:, :],
                        wx[:, k * h + m * 128:k * h + (m + 1) * 128],
                        xn[:, :],
                        start=(k == 0), stop=(k == KT - 1))
            for m in range(KT):
                # cast fp32 PSUM -> bf16 px^T slice
                nc.scalar.copy(
                    pxT[:, m * PXW + PAD + n * ns_:m * PXW + PAD + (n + 1) * ns_],
                    pp[m][:, :])

    tmpp = tc.alloc_tile_pool(name="tmp", bufs=3)
    psp = tc.alloc_tile_pool(name="ps", bufs=4, space="PSUM")

    def inp_slice(i, c, hh):
        d = (w - 1) - i
        col0 = hh * PXW + PAD + c * CW - d
        return pxT[:, col0:col0 + CW]

    def stb(buf, c, hh):
        return buf[:, hh * s + c * CW:hh * s + (c + 1) * CW]

    # --- step 0 (state == 0): state = relu(g0*(inp + by_c)) ---------------
    # p0 = g0, q0 = g0*by_c per-partition scalars (host-precomputed from by).
    for c in range(NCH):
        for hh in range(KT):
            u0 = tmpp.tile([128, CW], F32, tag="tB")
            nc.vector.tensor_scalar(u0[:, :], inp_slice(0, c, hh),
                                    p0[:, hh:hh + 1], q0[:, hh:hh + 1],
                                    op0=OP.mult, op1=OP.add)
            nc.vector.tensor_scalar(stb(sbufs[1], c, hh), u0[:, :], 0.0, None,
                                    op0=OP.max)

    # --- steps 1..W-1 ------------------------------------------------------
    for i in range(1, w):
        scur = sbufs[i % 2]
        snxt = sbufs[(i + 1) % 2]
        last = (i == w - 1)
        for c in range(NCH):
            for hh in range(KT):
                # gate half: m-tile = KT + hh of Wy
                psG = psp.tile([128, CW], F32, tag="ps")
                mg = KT + hh
                for n in range(NT):
                    for k in range(KT):
                        nc.tensor.matmul(
                            psG[:, n * ns_:(n + 1) * ns_],
                            wy[:, k * 2 * h + mg * 128:k * 2 * h + (mg + 1) * 128],
                            scur[:, k * s + c * CW + n * ns_:
                                 k * s + c * CW + (n + 1) * ns_],
                            start=(k == 0), stop=(k == KT - 1))
                sig = tmpp.tile([128, CW], F32, tag="tA")
                nc.scalar.activation(sig[:, :], psG[:, :], AF.Sigmoid,
                                     bias=byt[:, mg:mg + 1], scale=1.0)
                # g1 = relu(1.2*sig - 0.1)  (lower clip; upper clip fused below)
                nc.scalar.activation(sig[:, :], sig[:, :], AF.Relu,
                                     bias=cneg[:, 0:1], scale=1.2)

                # cand half: m-tile = hh
                psC = psp.tile([128, CW], F32, tag="ps")
                for n in range(NT):
                    for k in range(KT):
                        nc.tensor.matmul(
                            psC[:, n * ns_:(n + 1) * ns_],
                            wy[:, k * 2 * h + hh * 128:k * 2 * h + (hh + 1) * 128],
                            scur[:, k * s + c * CW + n * ns_:
                                 k * s + c * CW + (n + 1) * ns_],
                            start=(k == 0), stop=(k == KT - 1))
                u = tmpp.tile([128, CW], F32, tag="tB")
                # u = (cand + by_c) + inp
                nc.vector.scalar_tensor_tensor(
                    u[:, :], psC[:, :], byt[:, hh:hh + 1], inp_slice(i, c, hh),
                    op0=OP.add, op1=OP.add)
                # u = u - state
                nc.vector.tensor_tensor(u[:, :], u[:, :], stb(scur, c, hh),
                                        OP.subtract)
                # u = min(g1, 1) * u
                nc.vector.scalar_tensor_tensor(
                    u[:, :], sig[:, :], 1.0, u[:, :], op0=OP.min, op1=OP.mult)
                # u = u + state
                nc.vector.tensor_tensor(u[:, :], u[:, :], stb(scur, c, hh),
                                        OP.add)
                if not last:
                    nc.vector.tensor_scalar(stb(snxt, c, hh), u[:, :], 0.0,
                                            None, op0=OP.max)
                else:
                    fout = tmpp.tile([128, CW], F32, tag="tF", bufs=2)
                    nc.scalar.activation(fout[:, :], u[:, :], AF.Relu)
                    nc.sync.dma_start(
                        out_d[hh * 128:(hh + 1) * 128, c * CW:(c + 1) * CW],
                        fout[:, :])

    tmpp.release()
    psp.release()
    pers.release()


def build_program(s=S, h=H, w=W, nch=NCH, ns=NS):
    nc = bacc.Bacc("TRN2", target_bir_lowering=False, debug=False)
    xT = nc.dram_tensor("xT", [h, s], BF16, kind="ExternalInput")
    wx_d = nc.dram_tensor("Wx", [h, h], BF16, kind="ExternalInput")
    wy_d = nc.dram_tensor("Wy", [h, 2 * h], BF16, kind="ExternalInput")
    byt_d = nc.dram_tensor("byt", [128, 2 * h // 128], F32, kind="ExternalInput")
    p0_d = nc.dram_tensor("p0", [128, h // 128], F32, kind="ExternalInput")
    q0_d = nc.dram_tensor("q0", [128, h // 128], F32, kind="ExternalInput")
    out_d = nc.dram_tensor("out", [h, s], F32, kind="ExternalOutput")
    with tile.TileContext(nc) as tc:
        emit(nc, tc, s=s, h=h, w=w, nch=nch, ns=ns, xT=xT, wx_d=wx_d,
             wy_d=wy_d, byt_d=byt_d, p0_d=p0_d, q0_d=q0_d, out_d=out_d)
    nc.compile()
    return nc


def make_in_maps(x, Wx, Wy, by, s=S, h=H, b=B):
    bf = ml_dtypes.bfloat16
    Wx_b = np.ascontiguousarray(Wx.astype(bf))
    Wy_b = np.ascontiguousarray(Wy.astype(bf))
    by = by.astype(np.float32)
    byt = np.ascontiguousarray(by.reshape(2 * h // 128, 128).T)
    by_c, by_g = by[:h], by[h:]
    g0 = np.clip(1.2 / (1.0 + np.exp(-by_g.astype(np.float64))) - 0.1, 0.0, 1.0)
    g0 = g0.astype(np.float32)
    p0 = np.ascontiguousarray(g0.reshape(h // 128, 128).T)
    q0 = np.ascontiguousarray((g0 * by_c).reshape(h // 128, 128).T)
    in_maps = []
    for c in range(b):
        xTc = np.ascontiguousarray(x[c].astype(bf).T)
        in_maps.append({"xT": xTc, "Wx": Wx_b, "Wy": Wy_b,
                        "byt": byt, "p0": p0, "q0": q0})
    return in_maps


_NC_CACHE = {}


def _get_nc():
    if "nc" not in _NC_CACHE:
        _NC_CACHE["nc"] = build_program()
    return _NC_CACHE["nc"]


def kernel(x, Wx, Wy, by, _trace=False):
    nc = _get_nc()
    in_maps = make_in_maps(np.asarray(x, np.float32), np.asarray(Wx, np.float32),
                           np.asarray(Wy, np.float32), np.asarray(by, np.float32))
    res = bass_utils.run_bass_kernel_spmd(
        nc, in_maps, core_ids=list(range(B)), trace=_trace)
    out = np.stack([np.asarray(r["out"], np.float32).T for r in res.results])
    if _trace:
        return out, res
    return out
